# revision 4
# baseline (speedup 1.0000x reference)
"""Trainium2 Bass kernel for nn_ConceptIntergation (histogram_binning).

Reference computation:
    counts[b,s,n] = sum_k one_hot(concepts[b,s,k], 129)[..., n]  (n < 128; 128 = padding)
    out[b,s,n,d]  = counts[b,s,n] * emb_table[n,d]

Strategy (data-parallel over batch, 8 cores; HBM-write roofline ~146us/core):
  - Each core handles B_LOC=8 batches -> 1600 (b,s) rows, output shard
    [1600, 128*64] f32 (~52 MB). All inputs arrive in ONE ~190KB DMA
    (iota | idx | emb | identity packed per-partition) -- no 4MB
    pre-broadcast table read.
  - The expansion out_block[rows, (n d)] = counts @ W runs on the TENSOR
    engine: W[n', n*64+d] = emb[n,d] * (n==n') is a block-diagonal
    [128, 8192] bf16 matrix built on-device by DVE tensor_tensor
    (emb broadcast x identity-column broadcast), and lhsT = countsT
    (PE transpose of the DVE histogram). Exactly one nonzero product per
    output element, so bf16 only rounds emb (rel err <= 2^-8), f32 PSUM.
  - PSUM->SBUF drains alternate ScalarE/VectorE. Stripe 0 uses 0.5MB
    stores to get the first store out by ~14us; stripes 1-3 use 1MB
    stores. W chunks build on DVE interleaved with the first histograms
    (GpSimd is avoided entirely: its SBUF port contention slows DVE 7x).
    Engine busy/core: PE ~110us, DVE ~80us, ScalarE ~65us -- all under
    the 146us DMA floor, so the store stream saturates end to end.
"""

import numpy as np

import concourse.bass as bass
import concourse.mybir as mybir
from concourse import bacc
from concourse.tile import TileContext
from concourse.bass_utils import run_bass_kernel_spmd

B, S, K = 64, 200, 4
N, D = 128, 64
ND = N * D                      # 8192
NCORES = 8
B_LOC = B // NCORES             # 8
ROWS = B_LOC * S                # 1600 (b,s) rows per core
P = 128
NBLK = (ROWS + P - 1) // P      # 13 (12 full + 1 of 64 rows)

CC = 4                          # W chunks / output column stripes
CW = ND // CC                   # 2048 cols per stripe
MW = CW // D                    # 32 n-rows per stripe
FD = 512                        # matmul moving free dim (1 PSUM bank f32)

# packed const input column offsets
C_IOTA = 0
C_IDX = C_IOTA + N              # 128
C_EMB = C_IDX + NBLK * K        # 180
C_ID = C_EMB + D                # 244
C_TOT = C_ID + P                # 372

F32 = mybir.dt.float32
BF16 = mybir.dt.bfloat16

_NC_CACHE = {}


def _build_nc():
    nc = bacc.Bacc()
    cst = nc.declare_dram_parameter("cst", [P, C_TOT], F32, isOutput=False)
    out = nc.declare_dram_parameter("out", [ROWS, ND], F32, isOutput=True)

    with TileContext(nc) as tc:
        with (
            tc.tile_pool(name="const", bufs=1) as cpool,
            tc.tile_pool(name="cnt", bufs=2) as cntpool,
            tc.tile_pool(name="cntT", bufs=NBLK) as ctpool,
            tc.tile_pool(name="work", bufs=6) as wpool,
            tc.tile_pool(name="work1", bufs=6) as wpool1,
            tc.tile_pool(name="psmm", bufs=3, space="PSUM") as pmm,
            tc.tile_pool(name="pstr", bufs=2, space="PSUM") as ptr,
        ):
            cst_sb = cpool.tile([P, C_TOT], F32)
            nc.sync.dma_start(out=cst_sb, in_=cst[:, :])
            iota_sb = cst_sb[:, C_IOTA : C_IOTA + N]
            idx_sb = cst_sb[:, C_IDX : C_IDX + NBLK * K]
            emb_sb = cst_sb[:, C_EMB : C_EMB + D]
            ident_sb = cst_sb[:, C_ID : C_ID + P]

            Wt = [
                cpool.tile([P, CW], BF16, tag=f"W{c}", name=f"W{c}")
                for c in range(CC)
            ]

            def build_w(c, lo, hi):
                # W[n, (m d)] = emb[n, d] * (n == c*MW + m) for m in [lo, hi)
                nc.vector.tensor_tensor(
                    out=Wt[c][:, lo * D : hi * D].rearrange(
                        "p (m d) -> p m d", d=D
                    ),
                    in0=emb_sb[:, None, :].broadcast_to([P, hi - lo, D]),
                    in1=ident_sb[:, c * MW + lo : c * MW + hi, None].broadcast_to(
                        [P, hi - lo, D]
                    ),
                    op=mybir.AluOpType.mult,
                )

            def emit_countsT(j, pj):
                counts = cntpool.tile([P, N], F32, tag="cnt")
                nc.vector.tensor_scalar(
                    out=counts[:pj],
                    in0=iota_sb[:pj],
                    scalar1=idx_sb[:pj, j * K : j * K + 1],
                    scalar2=None,
                    op0=mybir.AluOpType.is_equal,
                )
                for k in range(1, K):
                    nc.vector.scalar_tensor_tensor(
                        out=counts[:pj],
                        in0=iota_sb[:pj],
                        scalar=idx_sb[:pj, j * K + k : j * K + k + 1],
                        in1=counts[:pj],
                        op0=mybir.AluOpType.is_equal,
                        op1=mybir.AluOpType.add,
                    )
                pst = ptr.tile([P, P], F32, tag="pst")
                nc.tensor.transpose(
                    pst[:, :pj], counts[:pj, :], ident_sb[:pj, :pj]
                )
                ct = ctpool.tile([P, P], BF16, tag="ct")
                nc.scalar.activation(
                    ct[:, :pj], pst[:, :pj], mybir.ActivationFunctionType.Copy
                )
                return ct

            state = {"ncopy": 0}

            def drain_copy(dst, src):
                # ~60/40 ScalarE/VectorE split keeps both well under the
                # DMA floor (DVE also runs histograms + W builds)
                if state["ncopy"] % 5 != 1 and state["ncopy"] % 5 != 3:
                    nc.scalar.activation(
                        dst, src, mybir.ActivationFunctionType.Copy
                    )
                else:
                    nc.vector.tensor_copy(out=dst, in_=src)
                state["ncopy"] += 1

            def emit_half(cc, j, pj, ct, h, ot=None):
                """one [pj, 2*FD] output half-chunk: 2 matmuls + drain copy.
                Stores directly (0.5MB) if ot is None, else copies into ot."""
                ps = pmm.tile([P, 2 * FD], F32, tag="ps")
                for q in range(2):
                    nc.tensor.matmul(
                        ps[:pj, q * FD : (q + 1) * FD],
                        ct[:, :pj],
                        Wt[cc][:, (2 * h + q) * FD : (2 * h + q + 1) * FD],
                        start=True,
                        stop=True,
                    )
                if ot is None:
                    ot1 = wpool1.tile([P, 2 * FD], F32, tag="ot1")
                    drain_copy(ot1[:pj], ps[:pj])
                    nc.sync.dma_start(
                        out=out[
                            j * P : j * P + pj,
                            cc * CW + 2 * h * FD : cc * CW + 2 * (h + 1) * FD,
                        ],
                        in_=ot1[:pj],
                    )
                else:
                    drain_copy(ot[:pj, 2 * h * FD : 2 * (h + 1) * FD], ps[:pj])

            # Partial block (64 rows) first so its half-width DMAs overlap
            # the full-width stream instead of trailing it. W chunk builds
            # interleave with the first histograms on DVE: chunk 0 in
            # halves right before first use, chunks 1-3 during stripe 0.
            order = [NBLK - 1] + list(range(NBLK - 1))
            cts = [None] * NBLK
            for cc in range(CC):
                for oi, j in enumerate(order):
                    pj = min(P, ROWS - j * P)
                    if cc == 0:
                        cts[j] = emit_countsT(j, pj)
                        if oi == 0:
                            build_w(0, 0, MW // 2)
                        elif oi in (1, 2, 3):
                            build_w(oi, 0, MW)
                        # stripe 0: store per half (0.5MB) for a short
                        # first-store chain
                        emit_half(cc, j, pj, cts[j], 0)
                        if oi == 0:
                            build_w(0, MW // 2, MW)
                        emit_half(cc, j, pj, cts[j], 1)
                    else:
                        ot = wpool.tile([P, CW], F32, tag="ot")
                        for h in range(2):
                            emit_half(cc, j, pj, cts[j], h, ot=ot)
                        nc.sync.dma_start(
                            out=out[j * P : j * P + pj, cc * CW : (cc + 1) * CW],
                            in_=ot[:pj],
                        )

    nc.finalize()
    return nc


def _get_nc():
    if "nc" not in _NC_CACHE:
        _NC_CACHE["nc"] = _build_nc()
    return _NC_CACHE["nc"]


def _prepare_in_maps(concepts, emb_table):
    concepts = np.asarray(concepts)
    emb = np.asarray(emb_table, dtype=np.float32)

    # per-core index shards, padded to NBLK*P rows, laid out [P, NBLK*K]
    conc = concepts.reshape(NCORES, ROWS, K).astype(np.float32)
    idx_pad = np.full((NCORES, NBLK * P, K), float(N), dtype=np.float32)
    idx_pad[:, :ROWS] = conc
    # [core, NBLK, P, K] -> [core, P, NBLK*K]
    idx_dev = idx_pad.reshape(NCORES, NBLK, P, K).transpose(0, 2, 1, 3).reshape(
        NCORES, P, NBLK * K
    )

    cst = np.empty((NCORES, P, C_TOT), dtype=np.float32)
    cst[:, :, C_IOTA : C_IOTA + N] = np.arange(N, dtype=np.float32)
    cst[:, :, C_IDX : C_IDX + NBLK * K] = idx_dev
    cst[:, :, C_EMB : C_EMB + D] = emb
    cst[:, :, C_ID : C_ID + P] = np.eye(P, dtype=np.float32)
    cst = np.ascontiguousarray(cst)
    return [{"cst": cst[i]} for i in range(NCORES)]


def _run(concepts, emb_table, **spmd_kwargs):
    nc = _get_nc()
    in_maps = _prepare_in_maps(concepts, emb_table)
    res = run_bass_kernel_spmd(nc, in_maps, core_ids=list(range(NCORES)), **spmd_kwargs)
    out = np.concatenate(
        [res.results[i]["out"].reshape(B_LOC, S, N, D) for i in range(NCORES)],
        axis=0,
    )
    return out, res


def kernel(concepts, emb_table):
    out, _ = _run(concepts, emb_table)
    return out


# revision 6
# speedup vs baseline: 1.0487x; 1.0487x over previous
"""Trainium2 Bass kernel for nn_ConceptIntergation (histogram_binning).

Reference computation:
    counts[b,s,n] = sum_k one_hot(concepts[b,s,k], 129)[..., n]  (n < 128; 128 = padding)
    out[b,s,n,d]  = counts[b,s,n] * emb_table[n,d]

Strategy (data-parallel over batch, 8 cores; HBM-write roofline ~146us/core):
  - Each core handles B_LOC=8 batches -> 1600 (b,s) rows, output shard
    [1600, 128*64] f32 (~52 MB). All inputs arrive in ONE ~190KB DMA
    (iota | idx | emb | identity packed per-partition) -- no 4MB
    pre-broadcast table read.
  - The expansion out_block[rows, (n d)] = counts @ W runs on the TENSOR
    engine: W[n', n*64+d] = emb[n,d] * (n==n') is a block-diagonal
    [128, 8192] bf16 matrix built on-device by DVE tensor_tensor
    (emb broadcast x identity-column broadcast), and lhsT = countsT
    (PE transpose of the DVE histogram). Exactly one nonzero product per
    output element, so bf16 only rounds emb (rel err <= 2^-8), f32 PSUM.
  - PSUM->SBUF drains alternate ScalarE/VectorE. Stripe 0 uses 0.5MB
    stores to get the first store out by ~14us; stripes 1-3 use 1MB
    stores. W chunks build on DVE interleaved with the first histograms
    (GpSimd is avoided entirely: its SBUF port contention slows DVE 7x).
    Engine busy/core: PE ~110us, DVE ~80us, ScalarE ~65us -- all under
    the 146us DMA floor, so the store stream saturates end to end.
"""

import numpy as np

import concourse.bass as bass
import concourse.mybir as mybir
from concourse import bacc
from concourse.tile import TileContext
from concourse.bass_utils import run_bass_kernel_spmd

B, S, K = 64, 200, 4
N, D = 128, 64
ND = N * D                      # 8192
NCORES = 8
B_LOC = B // NCORES             # 8
ROWS = B_LOC * S                # 1600 (b,s) rows per core
P = 128
NBLK = (ROWS + P - 1) // P      # 13 (12 full + 1 of 64 rows)

CC = 4                          # W chunks / output column stripes
CW = ND // CC                   # 2048 cols per stripe
MW = CW // D                    # 32 n-rows per stripe
FD = 512                        # matmul moving free dim (1 PSUM bank f32)

# packed const input column offsets
C_IOTA = 0
C_IDX = C_IOTA + N              # 128
C_EMB = C_IDX + NBLK * K        # 180
C_ID = C_EMB + D                # 244
C_TOT = C_ID + P                # 372

F32 = mybir.dt.float32
BF16 = mybir.dt.bfloat16

_NC_CACHE = {}


def _build_nc():
    nc = bacc.Bacc()
    cst = nc.declare_dram_parameter("cst", [P, C_TOT], F32, isOutput=False)
    out = nc.declare_dram_parameter("out", [ROWS, ND], F32, isOutput=True)

    with TileContext(nc) as tc:
        with (
            tc.tile_pool(name="const", bufs=1) as cpool,
            tc.tile_pool(name="cnt", bufs=2) as cntpool,
            tc.tile_pool(name="cntT", bufs=NBLK) as ctpool,
            tc.tile_pool(name="work", bufs=6) as wpool,
            tc.tile_pool(name="work1", bufs=6) as wpool1,
            tc.tile_pool(name="psmm", bufs=3, space="PSUM") as pmm,
            tc.tile_pool(name="pstr", bufs=2, space="PSUM") as ptr,
        ):
            cst_sb = cpool.tile([P, C_TOT], F32)
            nc.sync.dma_start(out=cst_sb, in_=cst[:, :])

            # PE HAM warmup: ~3.4us of junk matmuls during the otherwise
            # dead input-load window flips the PE clock gate to 8/8 before
            # the first real matmuls (else they run 1.7x slower at 1.2GHz).
            junk = cpool.tile([P, P], BF16)
            nc.vector.memset(junk[:, :], 0.0)
            junkW = cpool.tile([P, FD], BF16)
            nc.vector.memset(junkW[:, :], 0.0)
            for _ in range(8):
                psw = ptr.tile([P, FD], F32, tag="pst")
                nc.tensor.matmul(
                    psw[:, :], junk[:, :], junkW[:, :], start=True, stop=True
                )
            iota_sb = cst_sb[:, C_IOTA : C_IOTA + N]
            idx_sb = cst_sb[:, C_IDX : C_IDX + NBLK * K]
            emb_sb = cst_sb[:, C_EMB : C_EMB + D]
            ident_sb = cst_sb[:, C_ID : C_ID + P]

            Wt = [
                cpool.tile([P, CW], BF16, tag=f"W{c}", name=f"W{c}")
                for c in range(CC)
            ]

            def build_w(c, lo, hi):
                # W[n, (m d)] = emb[n, d] * (n == c*MW + m) for m in [lo, hi)
                nc.vector.tensor_tensor(
                    out=Wt[c][:, lo * D : hi * D].rearrange(
                        "p (m d) -> p m d", d=D
                    ),
                    in0=emb_sb[:, None, :].broadcast_to([P, hi - lo, D]),
                    in1=ident_sb[:, c * MW + lo : c * MW + hi, None].broadcast_to(
                        [P, hi - lo, D]
                    ),
                    op=mybir.AluOpType.mult,
                )

            def emit_countsT(j, pj):
                counts = cntpool.tile([P, N], F32, tag="cnt")
                nc.vector.tensor_scalar(
                    out=counts[:pj],
                    in0=iota_sb[:pj],
                    scalar1=idx_sb[:pj, j * K : j * K + 1],
                    scalar2=None,
                    op0=mybir.AluOpType.is_equal,
                )
                for k in range(1, K):
                    nc.vector.scalar_tensor_tensor(
                        out=counts[:pj],
                        in0=iota_sb[:pj],
                        scalar=idx_sb[:pj, j * K + k : j * K + k + 1],
                        in1=counts[:pj],
                        op0=mybir.AluOpType.is_equal,
                        op1=mybir.AluOpType.add,
                    )
                pst = ptr.tile([P, P], F32, tag="pst")
                nc.tensor.transpose(
                    pst[:, :pj], counts[:pj, :], ident_sb[:pj, :pj]
                )
                ct = ctpool.tile([P, P], BF16, tag="ct")
                nc.scalar.activation(
                    ct[:, :pj], pst[:, :pj], mybir.ActivationFunctionType.Copy
                )
                return ct

            state = {"ncopy": 0}

            def drain_copy(dst, src):
                # ~60/40 ScalarE/VectorE split keeps both well under the
                # DMA floor (DVE also runs histograms + W builds)
                if state["ncopy"] % 5 != 1 and state["ncopy"] % 5 != 3:
                    nc.scalar.activation(
                        dst, src, mybir.ActivationFunctionType.Copy
                    )
                else:
                    nc.vector.tensor_copy(out=dst, in_=src)
                state["ncopy"] += 1

            def emit_half(cc, j, pj, ct, h, ot=None):
                """one [pj, 2*FD] output half-chunk: 2 matmuls + drain copy.
                Stores directly (0.5MB) if ot is None, else copies into ot."""
                ps = pmm.tile([P, 2 * FD], F32, tag="ps")
                for q in range(2):
                    nc.tensor.matmul(
                        ps[:pj, q * FD : (q + 1) * FD],
                        ct[:, :pj],
                        Wt[cc][:, (2 * h + q) * FD : (2 * h + q + 1) * FD],
                        start=True,
                        stop=True,
                    )
                if ot is None:
                    ot1 = wpool1.tile([P, 2 * FD], F32, tag="ot1")
                    drain_copy(ot1[:pj], ps[:pj])
                    nc.sync.dma_start(
                        out=out[
                            j * P : j * P + pj,
                            cc * CW + 2 * h * FD : cc * CW + 2 * (h + 1) * FD,
                        ],
                        in_=ot1[:pj],
                    )
                else:
                    drain_copy(ot[:pj, 2 * h * FD : 2 * (h + 1) * FD], ps[:pj])

            # Partial block (64 rows) first so its half-width DMAs overlap
            # the full-width stream instead of trailing it. W chunk builds
            # interleave with the first histograms on DVE: chunk 0 in
            # halves right before first use, chunks 1-3 during stripe 0.
            order = [NBLK - 1] + list(range(NBLK - 1))
            cts = [None] * NBLK
            for cc in range(CC):
                for oi, j in enumerate(order):
                    pj = min(P, ROWS - j * P)
                    if cc == 0:
                        cts[j] = emit_countsT(j, pj)
                        if oi == 0:
                            build_w(0, 0, MW // 2)
                        elif oi in (5, 8, 11):
                            build_w(oi // 3, 0, MW)
                        # stripe 0: store per half (0.5MB) for a short
                        # first-store chain
                        emit_half(cc, j, pj, cts[j], 0)
                        if oi == 0:
                            build_w(0, MW // 2, MW)
                        emit_half(cc, j, pj, cts[j], 1)
                    else:
                        ot = wpool.tile([P, CW], F32, tag="ot")
                        for h in range(2):
                            emit_half(cc, j, pj, cts[j], h, ot=ot)
                        nc.sync.dma_start(
                            out=out[j * P : j * P + pj, cc * CW : (cc + 1) * CW],
                            in_=ot[:pj],
                        )

    nc.finalize()
    return nc


def _get_nc():
    if "nc" not in _NC_CACHE:
        _NC_CACHE["nc"] = _build_nc()
    return _NC_CACHE["nc"]


def _prepare_in_maps(concepts, emb_table):
    concepts = np.asarray(concepts)
    emb = np.asarray(emb_table, dtype=np.float32)

    # per-core index shards, padded to NBLK*P rows, laid out [P, NBLK*K]
    conc = concepts.reshape(NCORES, ROWS, K).astype(np.float32)
    idx_pad = np.full((NCORES, NBLK * P, K), float(N), dtype=np.float32)
    idx_pad[:, :ROWS] = conc
    # [core, NBLK, P, K] -> [core, P, NBLK*K]
    idx_dev = idx_pad.reshape(NCORES, NBLK, P, K).transpose(0, 2, 1, 3).reshape(
        NCORES, P, NBLK * K
    )

    cst = np.empty((NCORES, P, C_TOT), dtype=np.float32)
    cst[:, :, C_IOTA : C_IOTA + N] = np.arange(N, dtype=np.float32)
    cst[:, :, C_IDX : C_IDX + NBLK * K] = idx_dev
    cst[:, :, C_EMB : C_EMB + D] = emb
    cst[:, :, C_ID : C_ID + P] = np.eye(P, dtype=np.float32)
    cst = np.ascontiguousarray(cst)
    return [{"cst": cst[i]} for i in range(NCORES)]


def _run(concepts, emb_table, **spmd_kwargs):
    nc = _get_nc()
    in_maps = _prepare_in_maps(concepts, emb_table)
    res = run_bass_kernel_spmd(nc, in_maps, core_ids=list(range(NCORES)), **spmd_kwargs)
    out = np.concatenate(
        [res.results[i]["out"].reshape(B_LOC, S, N, D) for i in range(NCORES)],
        axis=0,
    )
    return out, res


def kernel(concepts, emb_table):
    out, _ = _run(concepts, emb_table)
    return out
